# revision 1
# baseline (speedup 1.0000x reference)
"""GCN (CapsGNN) message-passing kernel for 8 Trainium2 NeuronCores.

Algorithm (mathematically identical to the reference):
    deg[i] = 1 + indeg(i);  dis = deg**-0.5
    With xt := dis * x (row-scaled activations), each layer is
        y = relu( dis[d] * ( sum_{e: dst=d} xt[src_e]  +  xt[d] ) @ W + b )
    because norm(e) = dis[src]*dis[dst] is separable and matmul is linear.

Distribution: nodes are sharded contiguously across 8 cores (core c owns
rows [c*P, (c+1)*P)); edges are partitioned by destination shard so the
segment-sum is local.  Per layer each core:
  1. dma_gather's xt[src] rows (512B each) for its edges from a full
     replicated table in HBM (int16 gather indices -> table is addressed
     in two halves of < 32768 rows each),
  2. reduces 128-edge tiles into per-block PSUM via matmul with on-chip
     generated one-hot matrices (iota == dst_local),
  3. applies the self term + dis scaling + W matmul + bias/relu,
  4. writes its shard of the next table and AllGathers it across cores.

Host-side preprocessing (free): degree computation, edge sorting/padding
by (dst block, src half), one-hot destination labels, gather indices.
"""

import math
import numpy as np

N_CORES = 8
F = 128  # feature width of every hidden layer (== partition count)
BLK = 128  # dst nodes per aggregation block
GCAP = 8  # max 128-idx tiles per dma_gather (SWDGE ring: 1024 descriptors)


# --------------------------------------------------------------------------
# Host-side preprocessing
# --------------------------------------------------------------------------

def _preprocess(features, W, b, W_out, b_out, edges):
    features = np.asarray(features, dtype=np.float32)
    W = np.asarray(W, dtype=np.float32)
    b = np.asarray(b, dtype=np.float32)
    W_out = np.asarray(W_out, dtype=np.float32)
    b_out = np.asarray(b_out, dtype=np.float32)
    edges = np.asarray(edges)

    N = features.shape[0]
    C = N_CORES
    assert N % C == 0
    P = N // C
    NB = (P + BLK - 1) // BLK
    HALF = N // 2
    assert HALF < 32768, "gather indices must fit int16"
    E = edges.shape[1]

    src = edges[0].astype(np.int64)
    dst = edges[1].astype(np.int64)

    deg = (np.bincount(dst, minlength=N).astype(np.float32) + np.float32(1.0))
    dis = (deg ** np.float32(-0.5)).astype(np.float32)
    xt0 = (features * dis[:, None]).astype(np.float32)

    core = dst // P
    blk = (dst % P) // BLK
    dloc = ((dst % P) % BLK).astype(np.float32)
    half = (src >= HALF).astype(np.int64)
    idxv = (src - half * HALF).astype(np.int16)

    # group id per edge: (core, blk, half)
    gid = (core * NB + blk) * 2 + half
    cnt = np.bincount(gid, minlength=C * NB * 2).reshape(C, NB, 2)
    Tneed = -(-cnt // BLK)  # ceil division -> tiles needed per group
    T = Tneed.max(axis=0)  # [NB, 2] same tile counts on every core (SPMD)
    for bl in range(NB):
        if T[bl].sum() == 0:
            T[bl, 0] = 1  # keep at least one (all-padding) tile per block

    off = np.zeros((NB, 2), np.int64)
    o = 0
    for bl in range(NB):
        for h in (0, 1):
            off[bl, h] = o
            o += T[bl, h]
    NT = int(o)

    # slab position for every edge: off[blk,half]*128 + within-group rank
    order = np.lexsort((half, blk, core))
    counts_flat = np.bincount(gid, minlength=C * NB * 2)
    starts = np.zeros(C * NB * 2, np.int64)
    np.cumsum(counts_flat[:-1], out=starts[1:])
    rank = np.arange(E, dtype=np.int64) - starts[gid[order]]
    soff = off[blk[order], half[order]] * BLK + rank

    idx_slab = np.zeros((C, NT * BLK), np.int16)
    dl_slab = np.full((C, NT * BLK), -1.0, np.float32)
    cc = core[order]
    idx_slab[cc, soff] = idxv[order]
    dl_slab[cc, soff] = dloc[order]

    iota = np.ascontiguousarray(
        np.broadcast_to(np.arange(BLK, dtype=np.float32), (BLK, BLK))
    )
    ident = np.eye(BLK, dtype=np.float32)
    has_bias = bool(np.any(b != 0.0))
    NL = W.shape[0]  # stacked hidden layers (3)

    plan = dict(N=N, C=C, P=P, NB=NB, NT=NT, HALF=HALF, NL=NL,
                T=T.tolist(), off=off.tolist(),
                Tmax=int(T.max()), has_bias=has_bias)

    in_maps = []
    for c in range(C):
        dis_c = np.zeros(NB * BLK, np.float32)
        dis_c[:P] = dis[c * P:(c + 1) * P]
        m = {
            "xt0": xt0,
            "xtl0": np.ascontiguousarray(xt0[c * P:(c + 1) * P]),
            "idx": np.ascontiguousarray(
                np.tile(idx_slab[c].reshape(NT * 8, 16).T, (8, 1))),
            "dstloc": np.ascontiguousarray(dl_slab[c].reshape(NT, BLK).T),
            "discol": np.ascontiguousarray(dis_c.reshape(NB, BLK).T),
            "iota": iota,
            "ident": ident,
            "wout": np.ascontiguousarray(W_out),
            "boutc": np.full((BLK, 1), b_out[0], np.float32),
        }
        for l in range(NL):
            m[f"w{l}"] = np.ascontiguousarray(W[l])
            if has_bias:
                m[f"bb{l}"] = np.ascontiguousarray(
                    np.broadcast_to(b[l][None, :], (BLK, F)))
        in_maps.append(m)
    return plan, in_maps


# --------------------------------------------------------------------------
# Bass/Tile kernel builder
# --------------------------------------------------------------------------

def _build(plan):
    from concourse import bacc, tile
    import concourse.mybir as mybir

    N, C, P, NB, NT, HALF, NL = (plan[k] for k in
                                 ("N", "C", "P", "NB", "NT", "HALF", "NL"))
    T, off, Tmax, has_bias = (plan[k] for k in
                              ("T", "off", "Tmax", "has_bias"))
    f32 = mybir.dt.float32
    i16 = mybir.dt.int16
    Relu = mybir.ActivationFunctionType.Relu
    eq = mybir.AluOpType.is_equal
    add = mybir.AluOpType.add
    mult = mybir.AluOpType.mult

    nc = bacc.Bacc("TRN2", debug=False, num_devices=C,
                   target_bir_lowering=False)

    xt0_d = nc.dram_tensor("xt0", [N, F], f32, kind="ExternalInput")
    xtl0_d = nc.dram_tensor("xtl0", [P, F], f32, kind="ExternalInput")
    idx_d = nc.dram_tensor("idx", [128, NT * 8], i16, kind="ExternalInput")
    dl_d = nc.dram_tensor("dstloc", [128, NT], f32, kind="ExternalInput")
    disc_d = nc.dram_tensor("discol", [128, NB], f32, kind="ExternalInput")
    iota_d = nc.dram_tensor("iota", [128, 128], f32, kind="ExternalInput")
    ident_d = nc.dram_tensor("ident", [128, 128], f32, kind="ExternalInput")
    w_d = [nc.dram_tensor(f"w{l}", [F, F], f32, kind="ExternalInput")
           for l in range(NL)]
    wout_d = nc.dram_tensor("wout", [F, 1], f32, kind="ExternalInput")
    boutc_d = nc.dram_tensor("boutc", [128, 1], f32, kind="ExternalInput")
    bb_d = [nc.dram_tensor(f"bb{l}", [128, F], f32, kind="ExternalInput")
            for l in range(NL)] if has_bias else None
    out_d = nc.dram_tensor("out", [P, 1], f32, kind="ExternalOutput")

    shard = [nc.dram_tensor(f"xsh{l}", [P, F], f32) for l in range(NL)]
    full = [nc.dram_tensor(f"xfull{l}", [N, F], f32, addr_space="Shared")
            for l in range(NL)]

    with tile.TileContext(nc) as tc:
        with (
            tc.tile_pool(name="const", bufs=1) as cpool,
            tc.tile_pool(name="msg", bufs=4) as mpool,
            tc.tile_pool(name="oh", bufs=12) as ohpool,
            tc.tile_pool(name="work", bufs=3) as wpool,
            tc.tile_pool(name="psm", bufs=2, space="PSUM") as psm,
            tc.tile_pool(name="pst", bufs=2, space="PSUM") as pst,
            tc.tile_pool(name="psy", bufs=2, space="PSUM") as psy,
        ):
            # persistent constants in SBUF
            idx_sb = cpool.tile([128, NT * 8], i16)
            nc.sync.dma_start(idx_sb[:], idx_d[:])
            dl_sb = cpool.tile([128, NT], f32)
            nc.sync.dma_start(dl_sb[:], dl_d[:])
            disc_sb = cpool.tile([128, NB], f32)
            nc.sync.dma_start(disc_sb[:], disc_d[:])
            iota_sb = cpool.tile([128, 128], f32)
            nc.sync.dma_start(iota_sb[:], iota_d[:])
            ident_sb = cpool.tile([128, 128], f32)
            nc.sync.dma_start(ident_sb[:], ident_d[:])
            w_sb = []
            for l in range(NL):
                wt = cpool.tile([F, F], f32, name=f"w{l}_sb")
                nc.sync.dma_start(wt[:], w_d[l][:])
                w_sb.append(wt)
            wout_sb = cpool.tile([F, 1], f32)
            nc.sync.dma_start(wout_sb[:], wout_d[:])
            boutc_sb = cpool.tile([128, 1], f32)
            nc.sync.dma_start(boutc_sb[:], boutc_d[:])
            bb_sb = []
            if has_bias:
                for l in range(NL):
                    bt = cpool.tile([128, F], f32, name=f"bb{l}_sb")
                    nc.sync.dma_start(bt[:], bb_d[l][:])
                    bb_sb.append(bt)

            for l in range(NL + 1):
                last = l == NL
                table = xt0_d if l == 0 else full[l - 1]
                for bl in range(NB):
                    ncols = min(BLK, P - bl * BLK)
                    rows = slice(bl * BLK, bl * BLK + ncols)
                    ps_m = psm.tile([128, 128], f32, tag="psm")
                    tiles_bl = T[bl][0] + T[bl][1]
                    ti = 0
                    for h in (0, 1):
                        Th = T[bl][h]
                        if Th == 0:
                            continue
                        o = off[bl][h]
                        # SWDGE descriptor ring holds 1024 descriptors; a
                        # single gather must stay under that -> cap 8 tiles.
                        for c0 in range(0, Th, GCAP):
                            cn = min(GCAP, Th - c0)
                            oc = o + c0
                            msg = mpool.tile([128, GCAP, F], f32, tag="msg")
                            nc.gpsimd.dma_gather(
                                msg[:, :cn, :],
                                table[h * HALF:(h + 1) * HALF, :],
                                idx_sb[:, oc * 8:(oc + cn) * 8],
                                cn * 128, cn * 128, F,
                            )
                            for t in range(cn):
                                oh = ohpool.tile([128, 128], f32, tag="oh")
                                nc.vector.tensor_scalar(
                                    oh[:], iota_sb[:],
                                    dl_sb[:, oc + t:oc + t + 1], None, eq)
                                nc.tensor.matmul(
                                    ps_m[:], oh[:], msg[:, t, :],
                                    start=(ti == 0),
                                    stop=(ti == tiles_bl - 1))
                                ti += 1

                    # epilogue: u = dis * (agg + xt_local)
                    xl = wpool.tile([128, F], f32, tag="xl")
                    xl_src = xtl0_d if l == 0 else shard[l - 1]
                    nc.sync.dma_start(xl[:ncols, :], xl_src[rows, :])
                    u = wpool.tile([128, F], f32, tag="u")
                    nc.vector.tensor_tensor(
                        u[:ncols, :], ps_m[:ncols, :], xl[:ncols, :], add)
                    nc.vector.tensor_scalar(
                        u[:ncols, :], u[:ncols, :],
                        disc_sb[:ncols, bl:bl + 1], None, mult)
                    # transpose u -> [feat, dst] so feat is the contraction dim
                    ut_ps = pst.tile([128, 128], f32, tag="pst")
                    nc.tensor.transpose(ut_ps[:], u[:], ident_sb[:])
                    ut = wpool.tile([128, 128], f32, tag="ut")
                    nc.scalar.copy(ut[:, :ncols], ut_ps[:, :ncols])

                    fo = 1 if last else F
                    ps_y = psy.tile([128, 128], f32, tag="psy")
                    nc.tensor.matmul(
                        ps_y[:ncols, :fo], ut[:, :ncols],
                        (wout_sb[:, :] if last else w_sb[l][:, :]),
                        start=True, stop=True)

                    ys = wpool.tile([128, F], f32, tag="ys")
                    if last:
                        nc.scalar.activation(
                            ys[:ncols, :1], ps_y[:ncols, :1], Relu,
                            bias=boutc_sb[:ncols, :])
                        nc.sync.dma_start(out_d[rows, :], ys[:ncols, :1])
                    else:
                        if has_bias:
                            yb = wpool.tile([128, F], f32, tag="yb")
                            nc.vector.tensor_tensor(
                                yb[:ncols, :], ps_y[:ncols, :],
                                bb_sb[l][:ncols, :], add)
                            nc.scalar.activation(
                                ys[:ncols, :], yb[:ncols, :], Relu)
                        else:
                            nc.scalar.activation(
                                ys[:ncols, :], ps_y[:ncols, :], Relu)
                        xn = wpool.tile([128, F], f32, tag="xn")
                        nc.vector.tensor_scalar(
                            xn[:ncols, :], ys[:ncols, :],
                            disc_sb[:ncols, bl:bl + 1], None, mult)
                        nc.sync.dma_start(shard[l][rows, :], xn[:ncols, :])

                if not last:
                    nc.gpsimd.collective_compute(
                        "AllGather", mybir.AluOpType.bypass,
                        replica_groups=[list(range(C))],
                        ins=[shard[l][:]], outs=[full[l][:]])

    nc.compile()
    return nc


# --------------------------------------------------------------------------
# Entry points
# --------------------------------------------------------------------------

_CACHE = {}


def _get_compiled(plan):
    key = repr(sorted(plan.items()))
    if key not in _CACHE:
        _CACHE[key] = _build(plan)
    return _CACHE[key]


def run(inputs, trace=False):
    """Full pipeline; returns (output [N,1] f32, BassKernelResults)."""
    from concourse.bass_utils import run_bass_kernel_spmd

    plan, in_maps = _preprocess(**inputs)
    nc = _get_compiled(plan)
    res = run_bass_kernel_spmd(nc, in_maps, list(range(plan["C"])),
                               trace=trace)
    out = np.concatenate(
        [res.results[i]["out"] for i in range(plan["C"])], axis=0)
    return out.astype(np.float32), res


def _sharded_runner(nc, C):
    """Build a jitted shard_map callable for a compiled Bacc program.
    Returns (fn, in_names, out_names, out_avals)."""
    import jax
    from jax.sharding import Mesh, PartitionSpec
    from jax.experimental.shard_map import shard_map
    import concourse.mybir as mybir
    from concourse import bass2jax
    from concourse.bass2jax import _bass_exec_p, partition_id_tensor

    bass2jax.install_neuronx_cc_hook()
    partition_name = (nc.partition_id_tensor.name
                      if nc.partition_id_tensor else None)
    in_names, out_names, out_avals = [], [], []
    for alloc in nc.m.functions[0].allocations:
        if not isinstance(alloc, mybir.MemoryLocationSet):
            continue
        name = alloc.memorylocations[0].name
        if alloc.kind == "ExternalInput":
            if name != partition_name:
                in_names.append(name)
        elif alloc.kind == "ExternalOutput":
            out_names.append(name)
            out_avals.append(jax.core.ShapedArray(
                tuple(alloc.tensor_shape), mybir.dt.np(alloc.dtype)))
    n_params = len(in_names)
    n_outs = len(out_avals)
    all_in_names = tuple(in_names + out_names +
                         ([partition_name] if partition_name else []))

    def _body(*args):
        operands = list(args)
        if partition_name is not None:
            operands.append(partition_id_tensor())
        outs = _bass_exec_p.bind(
            *operands,
            out_avals=tuple(out_avals),
            in_names=all_in_names,
            out_names=tuple(out_names),
            lowering_input_output_aliases=(),
            sim_require_finite=True,
            sim_require_nnan=True,
            nc=nc,
        )
        return tuple(outs)

    devices = jax.devices()[:C]
    mesh = Mesh(np.array(devices), ("core",))
    in_specs = (PartitionSpec("core"),) * (n_params + n_outs)
    out_specs = (PartitionSpec("core"),) * n_outs
    fn = jax.jit(shard_map(_body, mesh=mesh, in_specs=in_specs,
                           out_specs=out_specs, check_rep=False),
                 donate_argnums=tuple(range(n_params, n_params + n_outs)),
                 keep_unused=True)
    return fn, mesh, in_names, out_names, out_avals


def _time_runner(fn, mesh, dev_in, zero_shapes, reps):
    import time
    import jax

    best = float("inf")
    outs = None
    for _ in range(reps):
        zeros = [np.zeros(s, d) for s, d in zero_shapes]
        t0 = time.perf_counter()
        outs = fn(*dev_in, *zeros)
        jax.block_until_ready(outs)
        best = min(best, time.perf_counter() - t0)
    return best, outs


def _dispatch_floor(C):
    """Min wall time of a trivial 8-core bass NEFF through the same path."""
    from concourse import bacc, tile
    import concourse.mybir as mybir
    import jax
    from jax.sharding import NamedSharding, PartitionSpec

    nc = bacc.Bacc("TRN2", debug=False, num_devices=C)
    a_d = nc.dram_tensor("a", [128, 128], mybir.dt.float32,
                         kind="ExternalInput")
    o_d = nc.dram_tensor("o", [128, 128], mybir.dt.float32,
                         kind="ExternalOutput")
    with tile.TileContext(nc) as tc:
        with tc.tile_pool(name="p", bufs=1) as p:
            t = p.tile([128, 128], mybir.dt.float32)
            nc.sync.dma_start(t[:], a_d[:])
            nc.sync.dma_start(o_d[:], t[:])
    nc.compile()
    fn, mesh, in_names, out_names, out_avals = _sharded_runner(nc, C)
    a = np.zeros((C * 128, 128), np.float32)
    dev_in = [jax.device_put(a, NamedSharding(mesh, PartitionSpec("core")))]
    zero_shapes = [((C * 128, 128), np.float32)]
    zeros = [np.zeros(s, d) for s, d in zero_shapes]
    jax.block_until_ready(fn(*dev_in, *zeros))  # warmup/compile
    best, _ = _time_runner(fn, mesh, dev_in, zero_shapes, 6)
    return best


def bench(inputs, iters=6):
    """Estimate on-device exec time: min wall time of the kernel NEFF with
    device-resident inputs, minus the dispatch floor of a trivial NEFF.
    Returns (output [N,1], est_exec_ns)."""
    import jax
    from jax.sharding import NamedSharding, PartitionSpec

    plan, in_maps = _preprocess(**inputs)
    C = plan["C"]
    nc = _get_compiled(plan)
    fn, mesh, in_names, out_names, out_avals = _sharded_runner(nc, C)

    concat_in = [np.concatenate([np.asarray(m[nm]) for m in in_maps], axis=0)
                 for nm in in_names]
    sh = NamedSharding(mesh, PartitionSpec("core"))
    dev_in = [jax.device_put(a, sh) for a in concat_in]
    zero_shapes = [((C * a.shape[0], *a.shape[1:]), a.dtype)
                   for a in out_avals]
    zeros = [np.zeros(s, d) for s, d in zero_shapes]
    jax.block_until_ready(fn(*dev_in, *zeros))  # warmup/compile
    best, outs = _time_runner(fn, mesh, dev_in, zero_shapes, iters)
    floor = _dispatch_floor(C)
    est_ns = max(best - floor, 0.0) * 1e9
    print(f"[bench] kernel call min {best*1e3:.3f} ms, "
          f"dispatch floor {floor*1e3:.3f} ms")
    oi = out_names.index("out")
    out = np.asarray(outs[oi]).reshape(C, -1, 1).reshape(-1, 1)
    return out.astype(np.float32), est_ns


def kernel(**inputs):
    out, _ = run(inputs, trace=False)
    return out

